# revision 2
# baseline (speedup 1.0000x reference)
"""CVQNN classifier kernel for 8 Trainium2 NeuronCores.

Math: the whole quantum circuit collapses to a batch-independent affine map
(S, d) on 128-dim phase space; per batch row
    msel = x @ W2 + d20          (W2 = S[rows, :64].T -> (64, 20))
    out_k = log1p(relu(msel_x[k]^2 + msel_p[k]^2 + cov_k/4 - 0.5))

Key structural fact: the 10 measured wires have a bounded light cone through
the nearest-neighbour beamsplitter circuit -- only ~16 of the 64 input
features have nonzero rows in W2 (dropped energy ~1e-19 for these params).
So the device only ever sees the top-16 features, in fp16 (rel_l2 ~4e-4,
worst elementwise ~8e-3 -- far inside the 2e-2 gate).

Device layout (per core, R = 126336 rows, J = 141 j-blocks):
  - host packs xs (113, J*128) fp16: partitions 16c+i = feature feats[i] of
    chunk c (7 row-chunks of J*128 rows each), partition 112 = constant 1.0.
  - one matmul per j-block: stationary = xs[:, 128j:128j+128] (113, 128),
    moving = wmov (113, 140) fp16 block-diagonal (chunk c -> psum cols
    20c..20c+19) with the d20 offset on the ones-partition row, so
    psum = x@W + d directly (no per-element DVE adds).
  - psum: 3 j-blocks per 512-col bank (420 used); super-block = 12 j = 4
    banks (tail SB 9 j = 3 banks), double-buffered = all 8 banks.
  - tail on device: sq = Square(psum) on ACT (fp16 out), pair-add
    sq_x + sq_p on DVE (fp16, 2x-packable) -> (128, 70j) -> gpsimd DMA out.
  - host finishes elementwise in exact fp32: out = log1p(relu(s + covc)).

Per-core DMA ~6.6 MB (4.1 in + 2.5 out) vs 37.3 MB for the fp32 baseline.
"""

import numpy as np

import concourse.bacc as bacc
import concourse.mybir as mybir
import concourse.tile as tile
from concourse.bass_utils import run_bass_kernel_spmd

N = 64          # wires
OUT = 10        # measured wires / classes
NCORES = 8
F = 16          # features shipped to the device (light cone of wires 0..9)
P = 7           # row-chunks packed per j-block (7*16 = 112 partitions + ones)
KP = P * F + 1  # stationary partitions (113)
J = 141         # j-blocks per core (12*11 + 9)
ROWS_J = P * 128            # batch rows per j-block (896)
R = J * ROWS_J              # per-core rows = 126336
B_PAD = R * NCORES          # 1010688
CPC = J * 128               # xs cols per core (18048)
WIDTHS = [12] * 11 + [9]    # j-blocks per super-block (4 / 3 psum banks)
F32 = mybir.dt.float32
F16 = mybir.dt.float16


# ---------------------------------------------------------------- host math
def _bs_pass(n, start, int_params):
    i = np.arange(start, n - 1, 2)
    j = i + 1
    theta = int_params[3 * i]
    phi = int_params[3 * i + 1]
    ct, st = np.cos(theta), np.sin(theta)
    cp, sp = np.cos(phi), np.sin(phi)
    S = np.eye(2 * n)
    S[i, i] = ct
    S[i, j] = -cp * st
    S[i, n + j] = -sp * st
    S[j, i] = cp * st
    S[j, j] = ct
    S[j, n + i] = -sp * st
    S[n + i, j] = sp * st
    S[n + i, n + i] = ct
    S[n + i, n + j] = -cp * st
    S[n + j, i] = sp * st
    S[n + j, n + i] = cp * st
    S[n + j, n + j] = ct
    return S


def _layer_symplectic(n, int1, squeezes, int2):
    M = _bs_pass(n, 0, int1)
    M = _bs_pass(n, 1, int1) @ M
    c = np.concatenate([np.cos(int1[2::3]), np.ones(1)])
    s = np.concatenate([np.sin(int1[2::3]), np.zeros(1)])
    Rm = np.block([[np.diag(c), np.diag(-s)], [np.diag(s), np.diag(c)]])
    Sq = np.diag(np.concatenate([np.exp(-squeezes), np.exp(squeezes)]))
    M = Sq @ (Rm @ M)
    M = _bs_pass(n, 0, int2) @ M
    M = _bs_pass(n, 1, int2) @ M
    return M


def _affine_map(layers):
    n = N
    S = np.eye(2 * n)
    d = np.zeros(2 * n)
    for int1, sq, int2, disp in layers:
        M = _layer_symplectic(n, int1, sq, int2)
        S = M @ S
        d = M @ d
        d[:n] += 2.0 * disp
    return S, d


def _device_constants(layers):
    S, d = _affine_map(layers)
    w = np.arange(OUT)
    rows = np.concatenate([w, N + w])
    cov = S @ S.T
    cov_term = cov[w, w] + cov[N + w, N + w]
    W2 = S[rows, :N].T                                   # (64, 20)
    d20 = (d[rows] / 2.0)
    covc = (cov_term / 4.0 - 0.5).astype(np.float32)     # (10,)
    feats = np.sort(np.argsort(-np.sum(W2 * W2, axis=1))[:F])

    wmov = np.zeros((KP, P * 2 * OUT), np.float16)       # (113, 140)
    for c in range(P):
        wmov[F * c:F * c + F, 20 * c:20 * c + 20] = W2[feats, :].astype(np.float16)
        wmov[KP - 1, 20 * c:20 * c + 20] = d20.astype(np.float16)
    return feats, wmov, covc


# ---------------------------------------------------------------- bass build
def build_nc(widths=None):
    widths = widths or WIDTHS
    jj = sum(widths)
    nc = bacc.Bacc("TRN2", target_bir_lowering=False)
    xs = nc.dram_tensor("xs", (KP, jj * 128), F16, kind="ExternalInput")
    wst = nc.dram_tensor("wmov", (KP, P * 2 * OUT), F16, kind="ExternalInput")
    out = nc.dram_tensor("out", (128, jj * P * OUT), F16, kind="ExternalOutput")

    Square = mybir.ActivationFunctionType.Square

    with tile.TileContext(nc) as tc:
        with (
            tc.tile_pool(name="const", bufs=1) as cpool,
            tc.tile_pool(name="xin", bufs=3) as xpool,
            tc.tile_pool(name="mid", bufs=3) as mpool,
            tc.tile_pool(name="ob", bufs=3) as opool,
            tc.tile_pool(name="ps", bufs=2, space="PSUM") as pspool,
        ):
            # w_t gates the first matmul: load it first on the sync queue
            w_t = cpool.tile([KP, P * 2 * OUT], F16)
            nc.sync.dma_start(w_t[:], wst[:])

            def emit_sb(j0, jblk, in_chunks):
                nbank = jblk // 3
                w = 128 * jblk
                col_base = j0 * 128
                tin = xpool.tile([KP, w], F16, tag="tin")
                q = w // in_chunks
                for c4 in range(in_chunks):
                    nc.sync.dma_start(
                        tin[:, c4 * q:(c4 + 1) * q],
                        xs[:, col_base + c4 * q:col_base + (c4 + 1) * q])

                # psum: 3 j-blocks use the first 420 cols of each 512-col bank
                ps = pspool.tile([128, nbank, 512], F32, tag="ps")
                for jl in range(jblk):
                    o = 140 * (jl % 3)
                    nc.tensor.matmul(
                        ps[:, jl // 3, o:o + 140],
                        tin[:, 128 * jl:128 * jl + 128], w_t[:],
                        start=True, stop=True,
                    )

                sq = mpool.tile([128, 140 * jblk], F16, tag="sq")
                nc.scalar.activation(sq[:], ps[:, :, 0:420], Square)

                sqv = sq[:].rearrange("p (j c r k) -> p j c r k",
                                      j=jblk, c=P, r=2, k=OUT)
                s = opool.tile([128, 70 * jblk], F16, tag="s")
                sv = s[:].rearrange("p (j c k) -> p j c k", j=jblk, c=P, k=OUT)
                nc.vector.tensor_add(sv, sqv[:, :, :, 0, :], sqv[:, :, :, 1, :])

                ob = j0 * 70
                nc.gpsimd.dma_start(out[:, ob:ob + 70 * jblk], s[:])

            # first tile's DMA in eighths so compute starts sooner
            j0 = 0
            for i, wdt in enumerate(widths):
                emit_sb(j0, wdt, 8 if i == 0 else 1)
                j0 += wdt
    nc.compile()
    return nc


# ---------------------------------------------------------------- host glue
def _make_in_maps(x_batch, feats, wmov):
    B = x_batch.shape[0]
    xpad = np.zeros((B_PAD, F), np.float16)
    xpad[:B] = x_batch[:, feats]
    in_maps = []
    for c in range(NCORES):
        xc = np.empty((KP, CPC), np.float16)
        blk = xpad[c * R:(c + 1) * R].reshape(P, CPC, F)
        xc[0:P * F] = blk.transpose(0, 2, 1).reshape(P * F, CPC)
        xc[P * F] = np.float16(1.0)
        in_maps.append({"xs": xc, "wmov": wmov})
    return in_maps


def _decode_out(results, B, covc):
    full = np.empty((B_PAD, OUT), np.float32)
    for c in range(NCORES):
        O = results[c]["out"].reshape(128, J, P, OUT)
        full[c * R:(c + 1) * R] = (
            O.transpose(2, 1, 0, 3).reshape(R, OUT))
    v = full[:B] + covc[None, :]
    return np.log1p(np.maximum(v, 0.0, out=v), out=v)


_NC_CACHE = {}


def kernel(x_batch, int1_0, squeezes_0, int2_0, disp_0,
           int1_1, squeezes_1, int2_1, disp_1, _trace=False):
    layers = [
        (np.asarray(int1_0, np.float64), np.asarray(squeezes_0, np.float64),
         np.asarray(int2_0, np.float64), np.asarray(disp_0, np.float64)),
        (np.asarray(int1_1, np.float64), np.asarray(squeezes_1, np.float64),
         np.asarray(int2_1, np.float64), np.asarray(disp_1, np.float64)),
    ]
    feats, wmov, covc = _device_constants(layers)
    in_maps = _make_in_maps(np.asarray(x_batch, np.float32), feats, wmov)

    if "nc" not in _NC_CACHE:
        _NC_CACHE["nc"] = build_nc()
    nc = _NC_CACHE["nc"]

    res = run_bass_kernel_spmd(
        nc, in_maps, core_ids=list(range(NCORES)), trace=_trace
    )
    out = _decode_out(res.results, x_batch.shape[0], covc)
    if _trace:
        return out, res
    return out


# revision 3
# speedup vs baseline: 4.0041x; 4.0041x over previous
"""CVQNN classifier kernel for 8 Trainium2 NeuronCores.

Math: the whole quantum circuit collapses to a batch-independent affine map
(S, d) on 128-dim phase space; per batch row
    msel = x @ W2 + d20          (W2 = S[rows, :64].T -> (64, 20))
    out_k = log1p(relu(msel_x[k]^2 + msel_p[k]^2 + cov_k/4 - 0.5))

Key structural fact: the 10 measured wires have a bounded light cone through
the nearest-neighbour beamsplitter circuit -- only ~16 of the 64 input
features have nonzero rows in W2 (dropped energy ~1e-19 for these params).
So the device only ever sees the top-16 features, in fp16 (rel_l2 ~4e-4,
worst elementwise ~8e-3 -- far inside the 2e-2 gate).

Device layout (per core, R = 126336 rows, J = 141 j-blocks):
  - host packs xs (113, J*128) fp16: partitions 16c+i = feature feats[i] of
    chunk c (7 row-chunks of J*128 rows each), partition 112 = constant 1.0.
  - one matmul per j-block: stationary = xs[:, 128j:128j+128] (113, 128),
    moving = wmov (113, 140) fp16 block-diagonal (chunk c -> psum cols
    20c..20c+19) with the d20 offset on the ones-partition row, so
    psum = x@W + d directly (no per-element DVE adds).
  - psum: 3 j-blocks per 512-col bank (420 used); super-block = 12 j = 4
    banks (tail SB 9 j = 3 banks), double-buffered = all 8 banks.
  - tail on device: sq = Square(psum) on ACT (fp16 out), pair-add
    sq_x + sq_p on DVE (fp16, 2x-packable) -> (128, 70j) -> gpsimd DMA out.
  - host finishes elementwise in exact fp32: out = log1p(relu(s + covc)).

Per-core DMA ~6.6 MB (4.1 in + 2.5 out) vs 37.3 MB for the fp32 baseline.
"""

import numpy as np

import concourse.bacc as bacc
import concourse.mybir as mybir
import concourse.tile as tile
from concourse.bass_utils import run_bass_kernel_spmd

N = 64          # wires
OUT = 10        # measured wires / classes
NCORES = 8
F = 16          # features shipped to the device (light cone of wires 0..9)
P = 7           # row-chunks packed per j-block (7*16 = 112 partitions + ones)
KP = P * F + 1  # meaningful stationary partitions (113)
KPD = 128       # padded partition count: HWDGE only fans a DMA across its
                # 16 SDMA rings when the partition dim divides evenly; a
                # 113-partition transfer serializes on one ring (~40 GB/s)
J = 141         # j-blocks per core (12*11 + 9)
ROWS_J = P * 128            # batch rows per j-block (896)
R = J * ROWS_J              # per-core rows = 126336
B_PAD = R * NCORES          # 1010688
CPC = J * 128               # xs cols per core (18048)
WIDTHS = [12] * 11 + [9]    # j-blocks per super-block (4 / 3 psum banks)
F32 = mybir.dt.float32
F16 = mybir.dt.float16


# ---------------------------------------------------------------- host math
def _bs_pass(n, start, int_params):
    i = np.arange(start, n - 1, 2)
    j = i + 1
    theta = int_params[3 * i]
    phi = int_params[3 * i + 1]
    ct, st = np.cos(theta), np.sin(theta)
    cp, sp = np.cos(phi), np.sin(phi)
    S = np.eye(2 * n)
    S[i, i] = ct
    S[i, j] = -cp * st
    S[i, n + j] = -sp * st
    S[j, i] = cp * st
    S[j, j] = ct
    S[j, n + i] = -sp * st
    S[n + i, j] = sp * st
    S[n + i, n + i] = ct
    S[n + i, n + j] = -cp * st
    S[n + j, i] = sp * st
    S[n + j, n + i] = cp * st
    S[n + j, n + j] = ct
    return S


def _layer_symplectic(n, int1, squeezes, int2):
    M = _bs_pass(n, 0, int1)
    M = _bs_pass(n, 1, int1) @ M
    c = np.concatenate([np.cos(int1[2::3]), np.ones(1)])
    s = np.concatenate([np.sin(int1[2::3]), np.zeros(1)])
    Rm = np.block([[np.diag(c), np.diag(-s)], [np.diag(s), np.diag(c)]])
    Sq = np.diag(np.concatenate([np.exp(-squeezes), np.exp(squeezes)]))
    M = Sq @ (Rm @ M)
    M = _bs_pass(n, 0, int2) @ M
    M = _bs_pass(n, 1, int2) @ M
    return M


def _affine_map(layers):
    n = N
    S = np.eye(2 * n)
    d = np.zeros(2 * n)
    for int1, sq, int2, disp in layers:
        M = _layer_symplectic(n, int1, sq, int2)
        S = M @ S
        d = M @ d
        d[:n] += 2.0 * disp
    return S, d


def _device_constants(layers):
    S, d = _affine_map(layers)
    w = np.arange(OUT)
    rows = np.concatenate([w, N + w])
    cov = S @ S.T
    cov_term = cov[w, w] + cov[N + w, N + w]
    W2 = S[rows, :N].T                                   # (64, 20)
    d20 = (d[rows] / 2.0)
    covc = (cov_term / 4.0 - 0.5).astype(np.float32)     # (10,)
    feats = np.sort(np.argsort(-np.sum(W2 * W2, axis=1))[:F])

    wmov = np.zeros((KPD, P * 2 * OUT), np.float16)      # (128, 140), rows 113+ zero
    for c in range(P):
        wmov[F * c:F * c + F, 20 * c:20 * c + 20] = W2[feats, :].astype(np.float16)
        wmov[KP - 1, 20 * c:20 * c + 20] = d20.astype(np.float16)
    return feats, wmov, covc


# ---------------------------------------------------------------- bass build
def build_nc(widths=None):
    widths = widths or WIDTHS
    jj = sum(widths)
    nc = bacc.Bacc("TRN2", target_bir_lowering=False)
    xs = nc.dram_tensor("xs", (KPD, jj * 128), F16, kind="ExternalInput")
    wst = nc.dram_tensor("wmov", (KPD, P * 2 * OUT), F16, kind="ExternalInput")
    out = nc.dram_tensor("out", (128, jj * P * OUT), F16, kind="ExternalOutput")

    Square = mybir.ActivationFunctionType.Square

    with tile.TileContext(nc) as tc:
        with (
            tc.tile_pool(name="const", bufs=1) as cpool,
            tc.tile_pool(name="xin", bufs=3) as xpool,
            tc.tile_pool(name="mid", bufs=3) as mpool,
            tc.tile_pool(name="ob", bufs=3) as opool,
            tc.tile_pool(name="ps", bufs=2, space="PSUM") as pspool,
        ):
            # w_t gates the first matmul: load it first on the sync queue
            w_t = cpool.tile([KPD, P * 2 * OUT], F16)
            nc.sync.dma_start(w_t[:], wst[:])

            def emit_sb(j0, jblk, in_chunks):
                nbank = jblk // 3
                w = 128 * jblk
                col_base = j0 * 128
                tin = xpool.tile([KPD, w], F16, tag="tin")
                q = w // in_chunks
                for c4 in range(in_chunks):
                    nc.sync.dma_start(
                        tin[:, c4 * q:(c4 + 1) * q],
                        xs[:, col_base + c4 * q:col_base + (c4 + 1) * q])

                # psum: 3 j-blocks use the first 420 cols of each 512-col bank
                ps = pspool.tile([128, nbank, 512], F32, tag="ps")
                for jl in range(jblk):
                    o = 140 * (jl % 3)
                    nc.tensor.matmul(
                        ps[:, jl // 3, o:o + 140],
                        tin[:, 128 * jl:128 * jl + 128], w_t[:],
                        start=True, stop=True,
                    )

                sq = mpool.tile([128, 140 * jblk], F16, tag="sq")
                nc.scalar.activation(sq[:], ps[:, :, 0:420], Square)

                sqv = sq[:].rearrange("p (j c r k) -> p j c r k",
                                      j=jblk, c=P, r=2, k=OUT)
                s = opool.tile([128, 70 * jblk], F16, tag="s")
                sv = s[:].rearrange("p (j c k) -> p j c k", j=jblk, c=P, k=OUT)
                nc.vector.tensor_add(sv, sqv[:, :, :, 0, :], sqv[:, :, :, 1, :])

                ob = j0 * 70
                nc.gpsimd.dma_start(out[:, ob:ob + 70 * jblk], s[:])

            # first tile's DMA in eighths so compute starts sooner
            j0 = 0
            for i, wdt in enumerate(widths):
                emit_sb(j0, wdt, 8 if i == 0 else 1)
                j0 += wdt
    nc.compile()
    return nc


# ---------------------------------------------------------------- host glue
def _make_in_maps(x_batch, feats, wmov):
    B = x_batch.shape[0]
    xpad = np.zeros((B_PAD, F), np.float16)
    xpad[:B] = x_batch[:, feats]
    in_maps = []
    for c in range(NCORES):
        xc = np.zeros((KPD, CPC), np.float16)
        blk = xpad[c * R:(c + 1) * R].reshape(P, CPC, F)
        xc[0:P * F] = blk.transpose(0, 2, 1).reshape(P * F, CPC)
        xc[P * F] = np.float16(1.0)
        in_maps.append({"xs": xc, "wmov": wmov})
    return in_maps


def _decode_out(results, B, covc):
    full = np.empty((B_PAD, OUT), np.float32)
    for c in range(NCORES):
        O = results[c]["out"].reshape(128, J, P, OUT)
        full[c * R:(c + 1) * R] = (
            O.transpose(2, 1, 0, 3).reshape(R, OUT))
    v = full[:B] + covc[None, :]
    return np.log1p(np.maximum(v, 0.0, out=v), out=v)


_NC_CACHE = {}


def kernel(x_batch, int1_0, squeezes_0, int2_0, disp_0,
           int1_1, squeezes_1, int2_1, disp_1, _trace=False):
    layers = [
        (np.asarray(int1_0, np.float64), np.asarray(squeezes_0, np.float64),
         np.asarray(int2_0, np.float64), np.asarray(disp_0, np.float64)),
        (np.asarray(int1_1, np.float64), np.asarray(squeezes_1, np.float64),
         np.asarray(int2_1, np.float64), np.asarray(disp_1, np.float64)),
    ]
    feats, wmov, covc = _device_constants(layers)
    in_maps = _make_in_maps(np.asarray(x_batch, np.float32), feats, wmov)

    if "nc" not in _NC_CACHE:
        _NC_CACHE["nc"] = build_nc()
    nc = _NC_CACHE["nc"]

    res = run_bass_kernel_spmd(
        nc, in_maps, core_ids=list(range(NCORES)), trace=_trace
    )
    out = _decode_out(res.results, x_batch.shape[0], covc)
    if _trace:
        return out, res
    return out
